# revision 4
# baseline (speedup 1.0000x reference)
"""CenterLoss on Trainium2 (Bass/Tile), 8-core data-parallel.

reference math:
    distmat[i, j] = ||x_i||^2 + ||c_j||^2 - 2 <x_i, c_j>
    dist[i] = distmat[i, labels[i]]  == ||x_i - c_{labels[i]}||^2
    loss = mean(clip(dist, 1e-12, 1e12))

Only the gathered rows centers[labels] matter, so each core:
  - takes a 256-sample shard of x / labels (two 128-row tiles packed
    side by side in the free dim; the host pre-swizzles the shard into
    that [128, 2*64] layout so device DMAs are contiguous)
  - indirect-DMA gathers the 256 matching center rows from the
    replicated centers table in HBM
  - computes sum ||x - c||^2 / 2048 on-chip (the clip at [1e-12, 1e12]
    never binds for N(0,1) data in 64 dims, so it is algebraically a
    no-op here; correctness is checked against the reference)
The host sums the 8 per-core partial means.
"""

import numpy as np

import concourse.bacc as bacc
import concourse.bass as bass
import concourse.mybir as mybir
from concourse.bass_utils import run_bass_kernel_spmd
from concourse.tile import TileContext

N_CORES = 8
BATCH = 2048
FEAT = 64
NUM_CLASSES = 100000
SHARD = BATCH // N_CORES  # 256 samples per core
P = 128
NT = SHARD // P  # 2 partition-tiles per core

_CACHE = {}


def _build_bass(single_gather: bool = False) -> bass.Bass:
    # single_gather=True (one indirect DMA with a [128, 2] offset AP) is
    # numerically correct in CoreSim but returns wrong rows on real HW —
    # verified by hw_gather.py. Keep the two-[128,1]-offset gathers.
    nc = bacc.Bacc()
    x = nc.dram_tensor("x", [P, NT * FEAT], mybir.dt.float32, kind="ExternalInput")
    labels = nc.dram_tensor("labels", [P, NT], mybir.dt.int32, kind="ExternalInput")
    centers = nc.dram_tensor(
        "centers", [NUM_CLASSES, FEAT], mybir.dt.float32, kind="ExternalInput"
    )
    out = nc.dram_tensor("out", [1, 1], mybir.dt.float32, kind="ExternalOutput")

    with TileContext(nc) as tc:
        with (
            tc.tile_pool(name="sb", bufs=1) as sb,
            tc.tile_pool(name="ps", bufs=1, space="PSUM") as ps,
        ):
            xt = sb.tile([P, NT * FEAT], mybir.dt.float32)
            lt = sb.tile([P, NT], mybir.dt.int32)
            ct = sb.tile([P, NT * FEAT], mybir.dt.float32)
            df = sb.tile([P, NT * FEAT], mybir.dt.float32)
            sq = sb.tile([P, NT * FEAT], mybir.dt.float32)
            ones = sb.tile([P, 1], mybir.dt.float32)

            nc.sync.dma_start(out=xt[:], in_=x[:, :])
            nc.sync.dma_start(out=lt[:], in_=labels[:, :])
            # ones = x[:, 0:1]*0 + 1 — the matmul reduction vector; also
            # warms DVE's view of the x DMA sem so the subtract below only
            # needs the gather wait (one sync-wait slot per instruction).
            nc.vector.tensor_scalar(
                out=ones[:],
                in0=xt[:, 0:1],
                scalar1=0.0,
                scalar2=1.0,
                op0=mybir.AluOpType.mult,
                op1=mybir.AluOpType.add,
            )
            if single_gather:
                # ct[p, t*64:(t+1)*64] = centers[lt[p, t]]
                nc.gpsimd.indirect_dma_start(
                    out=ct[:].rearrange("p (t f) -> p t f", f=FEAT),
                    out_offset=None,
                    in_=centers[:],
                    in_offset=bass.IndirectOffsetOnAxis(ap=lt[:, :], axis=0),
                )
            else:
                for t in range(NT):
                    nc.gpsimd.indirect_dma_start(
                        out=ct[:, t * FEAT : (t + 1) * FEAT],
                        out_offset=None,
                        in_=centers[:],
                        in_offset=bass.IndirectOffsetOnAxis(
                            ap=lt[:, t : t + 1], axis=0
                        ),
                    )

            nc.vector.tensor_tensor(
                out=df[:], in0=xt[:], in1=ct[:], op=mybir.AluOpType.subtract
            )
            nc.vector.tensor_tensor(
                out=sq[:], in0=df[:], in1=df[:], op=mybir.AluOpType.mult
            )
            dist_pp = sb.tile([P, 1], mybir.dt.float32)
            nc.vector.reduce_sum(out=dist_pp[:], in_=sq[:], axis=mybir.AxisListType.X)
            # cross-partition sum via ones^T @ dist_pp -> [1, 1]
            acc = ps.tile([1, 1], mybir.dt.float32, space="PSUM")
            nc.tensor.matmul(
                out=acc[:], lhsT=ones[:], rhs=dist_pp[:], start=True, stop=True
            )
            total = sb.tile([1, 1], mybir.dt.float32)
            # partial of the global mean: sum(shard) / BATCH
            nc.scalar.mul(out=total[:], in_=acc[:], mul=1.0 / BATCH)
            nc.sync.dma_start(out=out[:, :], in_=total[:])
    nc.compile()
    return nc


def _make_in_maps(x, labels, centers):
    x = np.ascontiguousarray(np.asarray(x, dtype=np.float32))
    centers = np.ascontiguousarray(np.asarray(centers, dtype=np.float32))
    labels_i32 = np.asarray(labels).astype(np.int32).reshape(BATCH)
    in_maps = []
    for i in range(N_CORES):
        xs = x[i * SHARD : (i + 1) * SHARD]
        ls = labels_i32[i * SHARD : (i + 1) * SHARD]
        in_maps.append(
            {
                # [256, 64] -> [128, 2*64]: column t*64+f = sample t*128+p
                "x": np.ascontiguousarray(
                    xs.reshape(NT, P, FEAT).transpose(1, 0, 2).reshape(P, NT * FEAT)
                ),
                # [256] -> [128, 2]: column t = label of sample t*128+p
                "labels": np.ascontiguousarray(ls.reshape(NT, P).transpose(1, 0)),
                "centers": centers,
            }
        )
    return in_maps


def kernel(x: np.ndarray, labels: np.ndarray, centers: np.ndarray) -> np.ndarray:
    if "nc" not in _CACHE:
        _CACHE["nc"] = _build_bass()
    nc = _CACHE["nc"]

    in_maps = _make_in_maps(x, labels, centers)
    res = run_bass_kernel_spmd(nc, in_maps, core_ids=list(range(N_CORES)))
    total = np.float32(0.0)
    for r in res.results:
        total += r["out"][0, 0]
    return np.asarray(total, dtype=np.float32)


# revision 6
# speedup vs baseline: 1.1747x; 1.1747x over previous
"""CenterLoss on Trainium2 (raw Bass, 8-core data-parallel).

reference math:
    distmat[i, j] = ||x_i||^2 + ||c_j||^2 - 2 <x_i, c_j>   (B=2048, C=100000)
    dist[i] = distmat[i, labels[i]]  == ||x_i - c_{labels[i]}||^2
    loss = mean(clip(dist, 1e-12, 1e12))

Only the gathered rows centers[labels] matter, so each core takes a
256-sample shard (two 128-row half-shards packed side by side in the free
dim; the host pre-swizzles the shard so device DMAs are contiguous) and:

  SP  : labels DMA -> [128, 2] int32 SBUF
  Pool: two indirect-DMA gathers (128 rows each) of centers[labels] from
        the replicated table in HBM — one [128,1] offset column per gather
        (a single [128,2]-offset gather returns wrong rows on real HW)
  ACT : x DMA (second HWDGE ring, off the critical path), then per
        half-shard Square(df / sqrt(B)) with per-partition accumulate
  DVE : df = x - c, per half-shard as soon as its gather lands
  SP  : dist_pp [128, 2] -> out

Half-shard t is subtracted/squared while gather t+1 is still in flight.
The host sums the 8x128x2 partials — together with the 8-way shard split
this is the "unshard" step. The clip at [1e-12, 1e12] never binds for
N(0,1) data in 64 dims (dist ~ chi^2 with mean 128; min over 2048 draws
is far above 1e-12), so it is algebraically a no-op here; correctness is
checked against the reference.
"""

import numpy as np

import concourse.bacc as bacc
import concourse.bass as bass
import concourse.mybir as mybir
from concourse.bass_utils import run_bass_kernel_spmd

N_CORES = 8
BATCH = 2048
FEAT = 64
NUM_CLASSES = 100000
SHARD = BATCH // N_CORES  # 256 samples per core
P = 128
NT = SHARD // P  # 2 half-shards per core

_CACHE = {}


def _build_bass() -> bass.Bass:
    nc = bacc.Bacc()
    x = nc.dram_tensor("x", [P, NT * FEAT], mybir.dt.float32, kind="ExternalInput")
    labels = nc.dram_tensor("labels", [P, NT], mybir.dt.int32, kind="ExternalInput")
    centers = nc.dram_tensor(
        "centers", [NUM_CLASSES, FEAT], mybir.dt.float32, kind="ExternalInput"
    )
    out = nc.dram_tensor("out", [P, NT], mybir.dt.float32, kind="ExternalOutput")

    with (
        nc.sbuf_tensor([P, NT * FEAT], mybir.dt.float32) as xt,
        nc.sbuf_tensor([P, NT], mybir.dt.int32) as lt,
        nc.sbuf_tensor([P, NT * FEAT], mybir.dt.float32) as ct,
        nc.sbuf_tensor([P, NT * FEAT], mybir.dt.float32) as df,
        nc.sbuf_tensor([P, NT * FEAT], mybir.dt.float32) as sq,
        nc.sbuf_tensor([P, NT], mybir.dt.float32) as dist_pp,
        nc.semaphore() as s_x,
        nc.semaphore() as s_l,
        nc.semaphore() as s_g0,
        nc.semaphore() as s_g1,
        nc.semaphore() as s_v,
        nc.semaphore() as s_sq,
        nc.semaphore() as s_out,
        nc.Block() as block,
    ):
        gather_sems = (s_g0, s_g1)

        @block.sync
        def _(sync: bass.BassEngine):
            sync.dma_start(out=lt[:], in_=labels[:, :]).then_inc(s_l, 16)
            sync.wait_ge(s_sq, NT)
            sync.dma_start(out=out[:, :], in_=dist_pp[:]).then_inc(s_out, 16)

        @block.gpsimd
        def _(g: bass.BassEngine):
            g.wait_ge(s_l, 16)
            for t, s_gt in enumerate(gather_sems):
                g.indirect_dma_start(
                    out=ct[:, t * FEAT : (t + 1) * FEAT],
                    out_offset=None,
                    in_=centers[:],
                    in_offset=bass.IndirectOffsetOnAxis(ap=lt[:, t : t + 1], axis=0),
                ).then_inc(s_gt, 16)

        @block.vector
        def _(v: bass.BassEngine):
            v.wait_ge(s_x, 16)
            for t, s_gt in enumerate(gather_sems):
                v.wait_ge(s_gt, 16)
                sl = slice(t * FEAT, (t + 1) * FEAT)
                v.tensor_tensor(
                    out=df[:, sl],
                    in0=xt[:, sl],
                    in1=ct[:, sl],
                    op=mybir.AluOpType.subtract,
                ).then_inc(s_v, 1)

        @block.scalar
        def _(s: bass.BassEngine):
            s.dma_start(out=xt[:], in_=x[:, :]).then_inc(s_x, 16)
            for t in range(NT):
                s.wait_ge(s_v, t + 1)
                sl = slice(t * FEAT, (t + 1) * FEAT)
                # dist_pp[p, t] = sum_f Square(df[p, f] / sqrt(B)) — the
                # per-(partition, half-shard) partial of the global mean.
                s.activation(
                    out=sq[:, sl],
                    in_=df[:, sl],
                    func=mybir.ActivationFunctionType.Square,
                    scale=float(1.0 / BATCH**0.5),
                    accum_out=dist_pp[:, t : t + 1],
                ).then_inc(s_sq, 1)

    nc.compile()
    return nc


def _make_in_maps(x, labels, centers):
    x = np.ascontiguousarray(np.asarray(x, dtype=np.float32))
    centers = np.ascontiguousarray(np.asarray(centers, dtype=np.float32))
    labels_i32 = np.asarray(labels).astype(np.int32).reshape(BATCH)
    in_maps = []
    for i in range(N_CORES):
        xs = x[i * SHARD : (i + 1) * SHARD]
        ls = labels_i32[i * SHARD : (i + 1) * SHARD]
        in_maps.append(
            {
                # [256, 64] -> [128, 2*64]: column t*64+f = sample t*128+p
                "x": np.ascontiguousarray(
                    xs.reshape(NT, P, FEAT).transpose(1, 0, 2).reshape(P, NT * FEAT)
                ),
                # [256] -> [128, 2]: column t = label of sample t*128+p
                "labels": np.ascontiguousarray(ls.reshape(NT, P).transpose(1, 0)),
                "centers": centers,
            }
        )
    return in_maps


def kernel(x: np.ndarray, labels: np.ndarray, centers: np.ndarray) -> np.ndarray:
    if "nc" not in _CACHE:
        _CACHE["nc"] = _build_bass()
    nc = _CACHE["nc"]

    in_maps = _make_in_maps(x, labels, centers)
    res = run_bass_kernel_spmd(nc, in_maps, core_ids=list(range(N_CORES)))
    total = np.float32(0.0)
    for r in res.results:
        total += np.sum(r["out"], dtype=np.float32)
    return np.asarray(total, dtype=np.float32)
